# revision 47
# baseline (speedup 1.0000x reference)
"""Trainium2 Bass kernel for nn_Attention (dense_transformer).

reference:
    weight = softmax(einsum("btd,bsd->bts", tgt, src), axis=1)   # over t!
    weight_sum = einsum("bts,bsd->btd", weight, src)
    output = concat([weight_sum, tgt], -1) @ W.T + b
    returns (output, weight)

Sharding: 8 cores = batch (4) x tgt-halves (2). Core c handles batch c//2,
tgt rows [1024*(c%2), 1024*(c%2+1)).

Per-core layout trick: scores are computed TRANSPOSED (s on partitions,
t on free axis) so the softmax reduction (over t, cross-core) becomes a
free-axis reduce plus a tiny AllGather of per-column (max, sum) pairs
between the two cores sharing a batch. The exchange is split into two
halves so the first overlaps the second half of the score matmuls.

Precision: scores via fp16 hi/lo 3-pass matmuls (fp32-level accuracy,
1 cyc/row each) or single-pass float32r (11-bit mantissa); stages 2/3 in
fp16. PSUM accumulation is fp32 throughout.

All matmul operands are pre-transposed/blocked on the host so every DMA
is a contiguous linear copy.
"""
import numpy as np

B, S, D, O = 4, 2048, 1024, 1024
NC_ = 8
TL = S // 2          # t rows per core (1024)
SC = S // 128        # 16 s-chunks
DC = D // 128        # 8 d-chunks
OC = O // 128        # 8 o-chunks
TH = 2               # t halves of 512 within a core
TW = TL // TH        # 512
HS = SC // 2         # 8 s-chunks per collective half
PAIRS = [[0, 1], [2, 3], [4, 5], [6, 7]]

STAGE1 = "f32r"     # "fp16_3p" (exact, slower) or "f32r" (fast, ~11-bit scores)

_CACHE = {}


def _build(stage1=None):
    stage1 = stage1 or STAGE1
    key = ("nc", stage1)
    if key in _CACHE:
        return _CACHE[key]
    import concourse.bass as bass  # noqa: F401
    import concourse.mybir as mybir
    import concourse.tile as tile
    from concourse import bacc

    f32 = mybir.dt.float32
    f32r = mybir.dt.float32r
    fp16 = mybir.dt.float16

    nc = bacc.Bacc("TRN2", target_bir_lowering=False, debug=False, num_devices=NC_)

    # ---- per-core DRAM parameters (host pre-blocked, all linear DMAs) ----
    if stage1 == "fp16_3p":
        srct_hi_d = nc.dram_tensor("srct_hi", [SC, 128, DC, 128], fp16, kind="ExternalInput")
        srct_lo_d = nc.dram_tensor("srct_lo", [SC, 128, DC, 128], fp16, kind="ExternalInput")
        tgtt_hi_d = nc.dram_tensor("tgtt_hi", [128, DC, TL], fp16, kind="ExternalInput")
        tgtt_lo_d = nc.dram_tensor("tgtt_lo", [128, DC, TL], fp16, kind="ExternalInput")
    else:
        srct_r_d = nc.dram_tensor("srct_r", [SC, 128, DC, 128], f32r, kind="ExternalInput")
        tgtt_r_d = nc.dram_tensor("tgtt_r", [128, DC, TL], f32r, kind="ExternalInput")
    if stage1 != "fp16_3p":
        # tgt in fp16 for the final linear (rhs of W2 part); in fp16_3p
        # mode tgtt_hi already holds exactly these values.
        tgtt_f_d = nc.dram_tensor("tgtt_f", [128, DC, TL], fp16, kind="ExternalInput")
    src16_d = nc.dram_tensor("src16", [128, SC, D], fp16, kind="ExternalInput")
    wt16_d = nc.dram_tensor("wt16", [128, 2 * DC, O], fp16, kind="ExternalInput")
    bias_d = nc.dram_tensor("bias", [128, OC], f32, kind="ExternalInput")

    wout_d = nc.dram_tensor("wout", [S, TL], f32, kind="ExternalOutput")
    oout_d = nc.dram_tensor("oout", [O, TL], f32, kind="ExternalOutput")

    cc_in = [nc.dram_tensor(f"cc_in{h}", [128, 16], f32) for h in range(2)]
    cc_out = [nc.dram_tensor(f"cc_out{h}", [256, 16], f32) for h in range(2)]

    Exp = mybir.ActivationFunctionType.Exp
    AX = mybir.AxisListType.X
    MAX = mybir.AluOpType.max

    with tile.TileContext(nc) as tc:
        with tc.tile_pool(name="const", bufs=1) as cp, \
             tc.tile_pool(name="stream", bufs=4) as sp, \
             tc.tile_pool(name="wstage", bufs=3) as wp, \
             tc.tile_pool(name="ostage", bufs=3) as op, \
             tc.tile_pool(name="psum", bufs=6, space="PSUM") as pp:

            # ---- stage-1 rhs operands, DMA'd per d-chunk so the first
            #      matmul doesn't wait on full-tensor loads. First-needed
            #      pieces (d-chunk 0 + s-chunk 0 blocks) go first. ----
            first_stream = {}
            if stage1 == "fp16_3p":
                tgtt_hi = cp.tile([128, DC, TL], fp16, tag="tgtt_hi")
                tgtt_lo = cp.tile([128, DC, TL], fp16, tag="tgtt_lo")
                sh0 = sp.tile([128, DC, 128], fp16, tag="sh", name="sh0")
                sl0 = sp.tile([128, DC, 128], fp16, tag="sl", name="sl0")
                first_stream["hi"] = sh0
                first_stream["lo"] = sl0
                nc.sync.dma_start(tgtt_hi[:, 0, :], tgtt_hi_d[:, 0, :])
                nc.sync.dma_start(tgtt_lo[:, 0, :], tgtt_lo_d[:, 0, :])
                for dc in range(DC):
                    nc.sync.dma_start(sh0[:, dc, :], srct_hi_d[0, :, dc, :])
                    nc.sync.dma_start(sl0[:, dc, :], srct_lo_d[0, :, dc, :])
                for dc in range(1, DC):
                    nc.sync.dma_start(tgtt_hi[:, dc, :], tgtt_hi_d[:, dc, :])
                    nc.sync.dma_start(tgtt_lo[:, dc, :], tgtt_lo_d[:, dc, :])
            else:
                tgtt_r = cp.tile([128, DC, TL], f32r, tag="tgtt_r")
                sr0 = sp.tile([128, DC, 128], f32r, tag="sr", name="sr0")
                first_stream["r"] = sr0
                t0sl = slice(0, TW)
                # 2-chunk granularity: per-DMA issue cost (~0.65us on the
                # issuing sequencer) dominates small transfers; alternate
                # issue between sync and gpsimd so dispatch pipelines
                for g in range(DC // 2):
                    dsl = slice(2 * g, 2 * g + 2)
                    nc.sync.dma_start(tgtt_r[:, dsl, t0sl],
                                      tgtt_r_d[:, dsl, t0sl])
                    nc.sync.dma_start(sr0[:, dsl, :], srct_r_d[0, :, dsl, :])
                t1sl = slice(TW, TL)
                for g in range(2):
                    dsl = slice(4 * g, 4 * g + 4)
                    nc.sync.dma_start(tgtt_r[:, dsl, t1sl],
                                      tgtt_r_d[:, dsl, t1sl])

            # ---- PE warm-up: dummy matmuls on scratch data during the
            #      initial DMA fill. They absorb the cold-clock (HAM) ramp
            #      in otherwise-idle PE time so real matmuls run warm. ----
            warm = cp.tile([128, TW], fp16, tag="warm")
            nc.vector.memset(warm[:], 0.0)
            psw0 = pp.tile([128, TW], f32, tag="pswarm", bufs=1)
            for i in range(12):
                nc.tensor.matmul(psw0[:], warm[:, 0:128], warm[:],
                                 start=True, stop=True)

            expu = cp.tile([128, SC, TL], fp16, tag="expu")       # exp(s - m_l)
            m_all = cp.tile([128, SC, TH], f32, tag="m_all")      # per-tile col max
            sig_all = cp.tile([128, SC, TH], f32, tag="sig_all")  # per-tile exp sums
            # [m_l(8) | sig_l(8)] per collective half
            msig = [cp.tile([128, 16], f32, tag=f"msig{h}", name=f"msig{h}") for h in range(2)]
            mneg = cp.tile([128, SC], f32, tag="mneg")            # -m_l
            alpha = [cp.tile([128, HS], f32, tag=f"alpha{h}", name=f"alpha{h}") for h in range(2)]
            zero_b = cp.tile([128, 1], f32, tag="zero_b")
            nc.vector.memset(zero_b[:], 0.0)

            def half_exchange(h):
                """Emit (max,sum) pairwise exchange + alpha for s-chunk half h."""
                sl0 = slice(h * HS, (h + 1) * HS)
                nc.vector.tensor_add(msig[h][:, 8:16],
                                     sig_all[:, sl0, 0], sig_all[:, sl0, 1])
                nc.sync.dma_start(cc_in[h][:], msig[h][:])
                nc.gpsimd.collective_compute(
                    "AllGather", mybir.AluOpType.bypass,
                    ins=[cc_in[h][:]], outs=[cc_out[h][:]], replica_groups=PAIRS)
                ga = cp.tile([128, 16], f32, tag="ga", name=f"ga{h}")
                gb = cp.tile([128, 16], f32, tag="gb", name=f"gb{h}")
                nc.sync.dma_start(ga[:], cc_out[h][0:128, :])
                nc.sync.dma_start(gb[:], cc_out[h][128:256, :])
                mg = cp.tile([128, HS], f32, tag="mg", name=f"mg{h}")
                e0 = cp.tile([128, HS], f32, tag="e0", name=f"e0{h}")
                e1 = cp.tile([128, HS], f32, tag="e1", name=f"e1{h}")
                sg = cp.tile([128, HS], f32, tag="sg", name=f"sg{h}")
                tmp = cp.tile([128, HS], f32, tag="tmp", name=f"tmp{h}")
                nc.vector.tensor_max(mg[:], ga[:, 0:8], gb[:, 0:8])
                nc.vector.tensor_sub(tmp[:], ga[:, 0:8], mg[:])
                nc.scalar.activation(e0[:], tmp[:], Exp, bias=zero_b[:])
                nc.vector.tensor_sub(tmp[:], gb[:, 0:8], mg[:])
                nc.scalar.activation(e1[:], tmp[:], Exp, bias=zero_b[:])
                nc.vector.tensor_mul(e0[:], e0[:], ga[:, 8:16])
                nc.vector.tensor_mul(e1[:], e1[:], gb[:, 8:16])
                nc.vector.tensor_add(sg[:], e0[:], e1[:])
                nc.vector.reciprocal(sg[:], sg[:])
                nc.vector.tensor_sub(tmp[:], msig[h][:, 0:8], mg[:])
                nc.scalar.activation(alpha[h][:], tmp[:], Exp, bias=zero_b[:])
                nc.vector.tensor_mul(alpha[h][:], alpha[h][:], sg[:])

            # ---------------- stage 1: scoresT + exp ----------------
            for sc in range(SC):
                h, hs = sc // HS, sc % HS
                if stage1 == "fp16_3p":
                    if sc == 0:
                        sh, sl = first_stream["hi"], first_stream["lo"]
                    else:
                        sh = sp.tile([128, DC, 128], fp16, tag="sh")
                        sl = sp.tile([128, DC, 128], fp16, tag="sl")
                        nc.sync.dma_start(sh[:], srct_hi_d[sc])
                        nc.sync.dma_start(sl[:], srct_lo_d[sc])
                else:
                    if sc == 0:
                        sr = first_stream["r"]
                    elif sc <= 3:
                        # finer arrival granularity while the ramp catches up
                        sr = sp.tile([128, DC, 128], f32r, tag="sr")
                        nc.sync.dma_start(sr[:, 0:4, :], srct_r_d[sc, :, 0:4, :])
                        nc.sync.dma_start(sr[:, 4:8, :], srct_r_d[sc, :, 4:8, :])
                    else:
                        sr = sp.tile([128, DC, 128], f32r, tag="sr")
                        nc.sync.dma_start(sr[:], srct_r_d[sc])
                ps_t = []
                for th in range(TH):
                    ps = pp.tile([128, TW], f32, tag="ps")
                    tsl = slice(th * TW, (th + 1) * TW)
                    if stage1 == "fp16_3p":
                        n_mm = 3 * DC
                        i = 0
                        for dc in range(DC):
                            nc.tensor.matmul(ps[:], sh[:, dc, :], tgtt_hi[:, dc, tsl],
                                             start=(i == 0), stop=(i == n_mm - 1)); i += 1
                            nc.tensor.matmul(ps[:], sh[:, dc, :], tgtt_lo[:, dc, tsl],
                                             start=False, stop=(i == n_mm - 1)); i += 1
                            nc.tensor.matmul(ps[:], sl[:, dc, :], tgtt_hi[:, dc, tsl],
                                             start=False, stop=(i == n_mm - 1)); i += 1
                    else:
                        for dc in range(DC):
                            nc.tensor.matmul(ps[:], sr[:, dc, :], tgtt_r[:, dc, tsl],
                                             start=(dc == 0), stop=(dc == DC - 1))
                    nc.vector.tensor_reduce(m_all[:, sc, th:th + 1], ps[:],
                                            axis=AX, op=MAX)
                    ps_t.append(ps)
                # combine the two halves' maxes -> m_l, -m_l
                nc.vector.tensor_max(msig[h][:, hs:hs + 1],
                                     m_all[:, sc, 0:1], m_all[:, sc, 1:2])
                nc.vector.tensor_scalar_mul(mneg[:, sc:sc + 1],
                                            msig[h][:, hs:hs + 1], -1.0)
                for th in range(TH):
                    tsl = slice(th * TW, (th + 1) * TW)
                    nc.scalar.activation(expu[:, sc, tsl], ps_t[th][:], Exp,
                                         bias=mneg[:, sc:sc + 1],
                                         accum_out=sig_all[:, sc, th:th + 1])
                if sc == HS - 1:
                    half_exchange(0)    # overlaps the second half of stage 1
            half_exchange(1)

            # ---- stage-2/3 stationary operands (scheduler pulls these
            #      DMAs into stage-1 idle bandwidth) ----
            if stage1 == "fp16_3p":
                tgtt_f = tgtt_hi
            else:
                tgtt_f = cp.tile([128, DC, TL], fp16, tag="tgtt_f")
                nc.sync.dma_start(tgtt_f[:], tgtt_f_d[:])
            src16 = cp.tile([128, SC, D], fp16, tag="src16")
            wt16 = cp.tile([128, 2 * DC, O], fp16, tag="wt16")
            bias_sb = cp.tile([128, OC], f32, tag="bias")
            for g in range(4):
                gs = slice(g * SC // 4, (g + 1) * SC // 4)
                nc.sync.dma_start(src16[:, gs, :], src16_d[:, gs, :])
                gw = slice(g * 2 * DC // 4, (g + 1) * 2 * DC // 4)
                nc.sync.dma_start(wt16[:, gw, :], wt16_d[:, gw, :])
            nc.sync.dma_start(bias_sb[:], bias_d[:])

            # ---- early W2 pass: outT partial = W2T.T @ tgtT + b ----
            # Independent of the softmax, so the PE chews through it while
            # the second (max,sum) exchange is in flight.
            oacc = cp.tile([128, OC, TL], fp16, tag="oacc")
            for th in range(TH):
                tsl = slice(th * TW, (th + 1) * TW)
                for oi in range(OC):
                    osl = slice(oi * 128, (oi + 1) * 128)
                    psw = pp.tile([128, TW], f32, tag="ps")
                    for di in range(DC):
                        nc.tensor.matmul(psw[:], wt16[:, DC + di, osl],
                                         tgtt_f[:, di, tsl],
                                         start=(di == 0), stop=(di == DC - 1))
                    nc.vector.tensor_scalar_add(oacc[:, oi, tsl], psw[:],
                                                bias_sb[:, oi:oi + 1])

            # ---------------- weight finalize (fp16 in-place scale) --------
            for sc in range(SC):
                h, hs = sc // HS, sc % HS
                for th in range(TH):
                    tsl = slice(th * TW, (th + 1) * TW)
                    nc.vector.tensor_scalar_mul(expu[:, sc, tsl],
                                                expu[:, sc, tsl],
                                                alpha[h][:, hs:hs + 1])

            # ---------------- stage 2: weight_sumT = src.T @ weightT ------
            wst16 = cp.tile([128, DC, TL], fp16, tag="wst16")
            for th in range(TH):
                tsl = slice(th * TW, (th + 1) * TW)
                for di in range(DC):
                    ps2 = pp.tile([128, TW], f32, tag="ps")
                    for sc in range(SC):
                        nc.tensor.matmul(
                            ps2[:], src16[:, sc, di * 128:(di + 1) * 128],
                            expu[:, sc, tsl],
                            start=(sc == 0), stop=(sc == SC - 1))
                    nc.vector.tensor_copy(wst16[:, di, tsl], ps2[:])

            # ---------------- weight f32 output (cast of final weights) ---
            for sc in range(SC):
                for th in range(TH):
                    tsl = slice(th * TW, (th + 1) * TW)
                    wst = wp.tile([128, TW], f32, tag="wst")
                    nc.vector.tensor_copy(wst[:], expu[:, sc, tsl])
                    nc.sync.dma_start(
                        wout_d[sc * 128:(sc + 1) * 128, tsl], wst[:])

            # ---------------- stage 3: outT = W1T.T@wsT + oacc ------------
            for th in range(TH):
                tsl = slice(th * TW, (th + 1) * TW)
                for oi in range(OC):
                    osl = slice(oi * 128, (oi + 1) * 128)
                    ps3 = pp.tile([128, TW], f32, tag="ps")
                    for di in range(DC):
                        nc.tensor.matmul(ps3[:], wt16[:, di, osl],
                                         wst16[:, di, tsl],
                                         start=(di == 0), stop=(di == DC - 1))
                    ot = op.tile([128, TW], f32, tag="ot")
                    nc.vector.tensor_add(ot[:], ps3[:], oacc[:, oi, tsl])
                    nc.sync.dma_start(oout_d[osl, tsl], ot[:])

    nc.compile()
    _CACHE[key] = nc
    return nc


def _to_f32r(x):
    u = np.ascontiguousarray(x, dtype=np.float32).view(np.uint32)
    t = u + (((u >> 12) & 1) + 0x7FF)
    return (t & 0xFFFFF000).astype(np.uint32).view(np.float32)


def _prep_inputs(src, tgt, W, b, stage1=None):
    """Host-side shard + layout prep. Returns in_maps for cores 0..7."""
    stage1 = stage1 or STAGE1
    f16 = np.float16
    src = np.ascontiguousarray(src, dtype=np.float32)
    tgt = np.ascontiguousarray(tgt, dtype=np.float32)
    W = np.ascontiguousarray(W, dtype=np.float32)
    b = np.ascontiguousarray(b, dtype=np.float32)

    wt = W.T.astype(f16)                                   # [2D, O]
    wt16 = np.ascontiguousarray(
        wt.reshape(2 * DC, 128, O).transpose(1, 0, 2))     # [128, 16, O]
    bias_h = np.ascontiguousarray(b.reshape(OC, 128).T)    # [128, OC]

    def blk_srct(x):   # [D, S] -> [SC, 128(d), DC, 128(s)]
        return np.ascontiguousarray(
            x.reshape(DC, 128, SC, 128).transpose(2, 1, 0, 3))

    def blk_tgtt(x):   # [D, TL] -> [128(d), DC, TL]
        return np.ascontiguousarray(
            x.reshape(DC, 128, TL).transpose(1, 0, 2))

    in_maps = []
    for c in range(NC_):
        bi, th = c // 2, c % 2
        srcb = src[bi]                                     # [S, D]
        srcT = srcb.T                                      # [D, S]
        tgts = tgt[bi][th * TL:(th + 1) * TL]              # [TL, D]
        tgtT = tgts.T                                      # [D, TL]

        m = {
            "src16": np.ascontiguousarray(
                srcb.astype(f16).reshape(SC, 128, D).transpose(1, 0, 2)),
            "wt16": wt16, "bias": bias_h,
        }
        if stage1 == "fp16_3p":
            hi = srcT.astype(f16)
            lo = (srcT - hi.astype(np.float32)).astype(f16)
            thi = tgtT.astype(f16)
            tlo = (tgtT - thi.astype(np.float32)).astype(f16)
            m.update(srct_hi=blk_srct(hi), srct_lo=blk_srct(lo),
                     tgtt_hi=blk_tgtt(thi), tgtt_lo=blk_tgtt(tlo))
        else:
            m.update(srct_r=blk_srct(_to_f32r(srcT)),
                     tgtt_r=blk_tgtt(_to_f32r(tgtT)),
                     tgtt_f=blk_tgtt(tgtT.astype(f16)))
        in_maps.append(m)
    return in_maps


def kernel(src, tgt, W, b):
    import time
    from concourse.bass_utils import run_bass_kernel_spmd

    nc = _build()
    in_maps = _prep_inputs(src, tgt, W, b)
    try:
        res = run_bass_kernel_spmd(nc, in_maps, list(range(NC_)))
    except Exception:
        # transient tunnel/worker hiccups occasionally kill a dispatch;
        # one retry recovers when the backend is still healthy
        time.sleep(5.0)
        res = run_bass_kernel_spmd(nc, in_maps, list(range(NC_)))

    output = np.empty((B, S, O), dtype=np.float32)
    weight = np.empty((B, S, S), dtype=np.float32)
    for c in range(NC_):
        bi, th = c // 2, c % 2
        r = res.results[c]
        weight[bi, th * TL:(th + 1) * TL, :] = r["wout"].T
        output[bi, th * TL:(th + 1) * TL, :] = r["oout"].T
    return (output, weight)
